# revision 10
# baseline (speedup 1.0000x reference)
"""EnhancedEMA Trainium2 kernel (v3).

Math (per batch b):
  m_b   = mean(x[b])                                  (scalar)
  h1    = relu(m_b * rowsum(w1) + b1)                 ([C2]; C2 = T/4 = 512)
  tw    = softmax(h1 @ w2.T + b2)                     ([T])
  a_t   = position_alpha[t] * tw[t]
  s_0   = x[b,0]; s_t = a_t x_t + (1-a_t) s_{t-1}

Linearized EMA: c_t = prod_{j<=t}(1-a_j), g_t = a_t/c_t, Q_t = sum g_k x_k,
y_t = c_t * (x0 + Q_t).  sum_t a_t = 0.3 so c stays in [~0.7, 1].

v3 structure (per core, BPC=4 batches, column layout [t_lo=128, ...]):
  - z = g*x computed in place over the x tile (DVE/Pool split; GpSimd cannot
    read PSUM, so it only gets SBUF-to-SBUF work).
  - Running prefixes R_tb = x0 + sum_{j<tb} colsum(z_j) for ALL 16 blocks
    computed into one [16,512] PSUM tile by 16 accumulating matmuls with
    static prefix-pattern weights (LJ[j][p,i] = 1 iff i > j), plus a
    rank-1 ones matmul adding x0 to every row.
    This removes the serial carry chain: every scan matmul is independent.
  - Carry: z_tb[0,:] += R_tb (bf16 SBUF add, 2x DVE mode).
  - Scan: ps = U.T @ z_tb (inclusive cumsum including R in row 0) so
    y = c * ps is a single ACT scale PSUM->bf16.
  - y written as bf16, widened to f32 on host (tolerance budget 2e-2).
  - Batch means: b0/b1 on PE (ones-matmul, hides in DMA-in shadow),
    b2/b3 via DVE free-dim reduce + GpSimd partition all-reduce.
  - Temporal-attention matmuls paired across 2 batches (rhs = 2 h1 cols).
  - DMA: 32 in / 32 out instructions; y-DMAs issued from the Pool sequencer
    (25ns vs 565ns on sync).
"""

import os
import numpy as np
import ml_dtypes

B, T, C = 32, 2048, 512
NCORES = 8
BPC = B // NCORES        # batches per core
NTB = T // 128           # 16 t-blocks
C2 = T // 4              # 512 hidden
NJB = C2 // 128          # 4 j-blocks

_CACHE = {}
LAST_RESULTS = None


def _build_bass():
    import concourse.bass as bass
    import concourse.bacc as bacc
    import concourse.tile as tile
    from concourse import mybir, bass_isa
    from contextlib import ExitStack

    f32 = mybir.dt.float32
    bf16 = mybir.dt.bfloat16
    AF = mybir.ActivationFunctionType
    ALU = mybir.AluOpType
    AX = mybir.AxisListType

    nc = bacc.Bacc("TRN2", target_bir_lowering=False)

    x_d = nc.dram_tensor("x", [BPC, T, C], bf16, kind="ExternalInput")
    w2t_d = nc.dram_tensor("w2t", [NJB, 128, T], bf16, kind="ExternalInput")
    prm_d = nc.dram_tensor("prm", [128, 2 * NJB + 2 * NTB], f32,
                           kind="ExternalInput")
    u_d = nc.dram_tensor("ut", [128, 128], bf16, kind="ExternalInput")
    lj_d = nc.dram_tensor("lj", [128, (NTB - 1) * NTB], bf16,
                          kind="ExternalInput")
    y_d = nc.dram_tensor("y", [BPC, T, C], bf16, kind="ExternalOutput")

    with ExitStack() as ctx:
        tc = ctx.enter_context(tile.TileContext(nc))
        consts = ctx.enter_context(tc.tile_pool(name="consts", bufs=1))
        xpool = ctx.enter_context(tc.tile_pool(name="xpool", bufs=BPC))
        ypool = ctx.enter_context(tc.tile_pool(name="ypool", bufs=3))
        rpool = ctx.enter_context(tc.tile_pool(name="rpool", bufs=4))
        small = ctx.enter_context(tc.tile_pool(name="small", bufs=4))
        coef = ctx.enter_context(tc.tile_pool(name="coef", bufs=4))
        pscan = ctx.enter_context(tc.tile_pool(name="pscan", bufs=3, space="PSUM"))
        psS = ctx.enter_context(tc.tile_pool(name="psS", bufs=2, space="PSUM"))
        pmean = ctx.enter_context(tc.tile_pool(name="pmean", bufs=1, space="PSUM"))
        psmall = ctx.enter_context(tc.tile_pool(name="psmall", bufs=2, space="PSUM"))

        # ---- constants ----
        NPRM = 2 * NJB + 2 * NTB
        u_sb = consts.tile([128, 128], bf16, name="u_sb")
        nc.sync.dma_start(out=u_sb, in_=u_d[:, :])
        lj_sb = consts.tile([128, (NTB - 1) * NTB], bf16, name="lj_sb")
        nc.sync.dma_start(out=lj_sb, in_=lj_d[:, :])
        w2t_sb = consts.tile([128, NJB, T], bf16, name="w2t_sb")
        nc.sync.dma_start(
            out=w2t_sb,
            in_=bass.AP(tensor=w2t_d[0].tensor, offset=0,
                        ap=[[T, 128], [128 * T, NJB], [1, T]]))
        prm_sb = consts.tile([128, NPRM], f32, name="prm_sb")
        nc.sync.dma_start(out=prm_sb, in_=prm_d[:, :])
        s1_sb = prm_sb[:, 0:NJB]
        b1_sb = prm_sb[:, NJB:2 * NJB]
        b2_sb = prm_sb[:, 2 * NJB:2 * NJB + NTB]
        pa_sb = prm_sb[:, 2 * NJB + NTB:2 * NJB + 2 * NTB]
        onesmat = consts.tile([128, 128], bf16, name="onesmat")
        nc.vector.memset(onesmat, 1.0)
        zeros16 = consts.tile([1, NTB], f32, name="zeros16")
        nc.vector.memset(zeros16, 0.0)

        # ---- stage all x loads (8 chunk DMAs per batch, 2 tb each) ----
        xbs = []
        for b in range(BPC):
            xb = xpool.tile([128, NTB * C], bf16, name=f"xb{b}", tag="xb")
            for q in range(8):
                src = x_d[b, q * 2 * 128:(q + 1) * 2 * 128, :]
                nc.sync.dma_start(
                    out=xb[:, q * 2 * C:(q + 1) * 2 * C],
                    in_=bass.AP(tensor=src.tensor, offset=src.offset,
                                ap=[[C, 128], [128 * C, 2], [1, C]]))
            xbs.append(xb)

        # ---- batch means: sm_b replicated on all partitions ----
        sms = []
        for b in range(BPC):
            sm = small.tile([128, 1], f32, name=f"sm{b}", tag="sm")
            if b < 2:
                pm = pmean.tile([128, C], f32, name=f"pm{b}", tag="pm")
                for tb in range(NTB):
                    nc.tensor.matmul(pm, lhsT=onesmat,
                                     rhs=xbs[b][:, tb * C:(tb + 1) * C],
                                     start=(tb == 0), stop=(tb == NTB - 1))
                smr = small.tile([128, 1], f32, name=f"smr{b}", tag="smr")
                nc.vector.reduce_sum(out=smr, in_=pm, axis=AX.X)
                nc.scalar.mul(sm, smr, 1.0 / float(T * C))
            else:
                rsum = small.tile([128, 1], f32, name=f"rsum{b}", tag="smr")
                nc.vector.reduce_sum(out=rsum, in_=xbs[b], axis=AX.X)
                tot = small.tile([128, 1], f32, name=f"tot{b}", tag="tot")
                nc.gpsimd.partition_all_reduce(
                    tot, rsum, channels=128, reduce_op=bass_isa.ReduceOp.add)
                nc.scalar.mul(sm, tot, 1.0 / float(T * C))
            sms.append(sm)

        for pair in range(BPC // 2):
            b0, b1 = 2 * pair, 2 * pair + 1

            # ---- phase B (paired): h1 -> logits for both batches ----
            h1t2 = small.tile([128, 2 * NJB], bf16, name=f"h1t2{pair}",
                              tag="h1t2")
            for bi, b in enumerate((b0, b1)):
                t1 = small.tile([128, NJB], f32, name=f"t1{b}", tag="t1")
                nc.vector.tensor_scalar_mul(t1, in0=s1_sb, scalar1=sms[b])
                nc.vector.tensor_add(t1, t1, b1_sb)
                nc.scalar.activation(h1t2[:, bi * NJB:(bi + 1) * NJB], t1,
                                     AF.Relu)

            plb2 = psmall.tile([128, 2 * NTB], f32, name=f"plb2{pair}",
                               tag="ps")
            for tb in range(NTB):
                for jb in range(NJB):
                    nc.tensor.matmul(
                        plb2[:, tb * 2:tb * 2 + 2],
                        lhsT=w2t_sb[:, jb, tb * 128:(tb + 1) * 128],
                        rhs=h1t2[:, jb::NJB],
                        start=(jb == 0),
                        stop=(jb == NJB - 1),
                    )

            # ---- phase C per batch: softmax -> a; c = exp(cumsum ln(1-a)) ----
            gcols, ccols = {}, {}
            for bi, b in enumerate((b0, b1)):
                lg = coef.tile([128, NTB], f32, name=f"lg{b}", tag="lg")
                nc.vector.tensor_add(lg, plb2[:, bi::2], b2_sb)
                e = coef.tile([128, NTB], bf16, name=f"e{b}", tag="e")
                nc.scalar.activation(e, lg, AF.Exp)
                pse = psmall.tile([128, NTB], f32, name=f"pse{b}", tag="ps")
                nc.tensor.matmul(pse, lhsT=onesmat, rhs=e, start=True,
                                 stop=True)
                esumt = small.tile([128, 1], f32, name=f"esumt{b}",
                                   tag="esumt")
                nc.vector.reduce_sum(out=esumt, in_=pse, axis=AX.X)
                rb = small.tile([128, 1], f32, name=f"rb{b}", tag="rb")
                nc.vector.reciprocal(rb, esumt)
                acol = coef.tile([128, NTB], f32, name=f"acol{b}", tag="acol")
                nc.vector.tensor_scalar_mul(acol, in0=e, scalar1=rb)
                nc.vector.tensor_mul(acol, acol, pa_sb)

                Lb = coef.tile([128, NTB], bf16, name=f"Lb{b}", tag="Lb")
                nc.scalar.activation(Lb, acol, AF.Ln, bias=1.0, scale=-1.0)
                nc.scalar.mul(Lb[0:1, 0:1], Lb[0:1, 0:1], 0.0)
                pT = psmall.tile([128, NTB], f32, name=f"pT{b}", tag="ps")
                nc.tensor.matmul(pT, lhsT=onesmat, rhs=Lb, start=True,
                                 stop=True)
                Tsb = small.tile([1, NTB], f32, name=f"Tsb{b}", tag="Tsb")
                nc.scalar.copy(Tsb, pT[0:1, :])
                stage = small.tile([128, NTB + 1], bf16, name=f"stage{b}",
                                   tag="stage")
                nc.vector.memset(stage, 0.0)
                nc.vector.tensor_tensor_scan(
                    out=stage[0:1, 1:NTB + 1], data0=Tsb, data1=zeros16,
                    initial=0.0, op0=ALU.add, op1=ALU.add,
                )
                pcs = psmall.tile([128, NTB], f32, name=f"pcs{b}", tag="ps")
                nc.tensor.matmul(pcs, lhsT=u_sb, rhs=Lb, start=True,
                                 stop=False)
                nc.tensor.matmul(pcs, lhsT=onesmat, rhs=stage[:, 0:NTB],
                                 start=False, stop=True)
                ccol = coef.tile([128, NTB], f32, name=f"ccol{b}", tag="ccol")
                nc.scalar.activation(ccol, pcs, AF.Exp)
                rccol = coef.tile([128, NTB], f32, name=f"rccol{b}",
                                  tag="rccol")
                nc.scalar.activation(rccol, pcs, AF.Exp, scale=-1.0)
                gcol = coef.tile([128, NTB], f32, name=f"gcol{b}", tag="gcol")
                nc.vector.tensor_mul(gcol, acol, rccol)
                nc.vector.memset(gcol[0:1, 0:1], 0.0)
                gcols[b], ccols[b] = gcol, ccol

            # ---- phase D-1 per batch: z in place, block sums S, prefixes R,
            #      carry rows into z ----
            for b in (b0, b1):
                xb = xbs[b]
                # x0 row (bf16) must be saved before z overwrites block 0
                x0row = rpool.tile([1, C], bf16, name=f"x0row{b}", tag="x0r")
                nc.scalar.copy(x0row, xb[0:1, 0:C])
                for tb in range(NTB):
                    sl = xb[:, tb * C:(tb + 1) * C]
                    gsc = gcols[b][:, tb:tb + 1]
                    if tb % 4 == 0:
                        nc.vector.tensor_scalar_mul(sl, in0=sl, scalar1=gsc)
                    else:
                        nc.gpsimd.tensor_scalar_mul(sl, in0=sl, scalar1=gsc)
                # R[i] = x0 + sum_{j<i} colsum(z_j), all 16 rows at once
                Rall = psS.tile([16, C], f32, name=f"Rall{b}", tag="SR")
                nc.tensor.matmul(Rall, lhsT=onesmat[0:1, 0:NTB], rhs=x0row,
                                 start=True, stop=False)
                for j in range(NTB - 1):
                    nc.tensor.matmul(Rall,
                                     lhsT=lj_sb[:, j * NTB:(j + 1) * NTB],
                                     rhs=xb[:, j * C:(j + 1) * C],
                                     start=False, stop=(j == NTB - 2))
                Rsb = rpool.tile([16, C], bf16, name=f"Rsb{b}", tag="Rsb")
                nc.vector.tensor_scalar_add(Rsb, in0=Rall, scalar1=0.0)
                # engines cannot address partition tb directly: flatten the
                # 16 R rows onto partition 0 with a tiny SBUF->SBUF DMA
                rrow = rpool.tile([1, NTB * C], bf16, name=f"rrow{b}",
                                  tag="rrow")
                nc.gpsimd.dma_start(out=rrow, in_=Rsb)
                for tb in range(NTB):
                    nc.vector.tensor_add(xb[0:1, tb * C:(tb + 1) * C],
                                         xb[0:1, tb * C:(tb + 1) * C],
                                         rrow[0:1, tb * C:(tb + 1) * C])

            # ---- phase D-2 (tb-major): scan matmuls + y scale + out DMA ----
            for tb in range(NTB):
                y2 = ypool.tile([128, 2 * C], bf16, name=f"y{pair}_{tb}",
                                tag="y")
                for bi, b in enumerate((b0, b1)):
                    ps = pscan.tile([128, C], f32, name=f"psQ{b}_{tb}",
                                    tag="s")
                    nc.tensor.matmul(ps, lhsT=u_sb,
                                     rhs=xbs[b][:, tb * C:(tb + 1) * C],
                                     start=True, stop=True)
                    nc.scalar.mul(y2[:, bi * C:(bi + 1) * C], ps,
                                  ccols[b][:, tb:tb + 1])
                dst = y_d[b0, tb * 128:(tb + 1) * 128, :]
                nc.gpsimd.dma_start(
                    out=bass.AP(tensor=dst.tensor, offset=dst.offset,
                                ap=[[C, 128], [T * C, 2], [1, C]]),
                    in_=y2)

    nc.compile()
    return nc


def _get_nc():
    if "nc" not in _CACHE:
        _CACHE["nc"] = _build_bass()
    return _CACHE["nc"]


def kernel(x, position_alpha, w1, b1, w2, b2):
    global LAST_RESULTS
    from concourse.bass_utils import run_bass_kernel_spmd

    x = np.asarray(x, dtype=np.float32)
    position_alpha = np.asarray(position_alpha, dtype=np.float32)
    w1 = np.asarray(w1, dtype=np.float32)
    b1 = np.asarray(b1, dtype=np.float32)
    w2 = np.asarray(w2, dtype=np.float32)
    b2 = np.asarray(b2, dtype=np.float32)

    # host-side parameter prep (layout only / trivial reductions)
    s1 = w1.sum(axis=1)                                   # [C2]
    s1r = s1.reshape(NJB, 128).T                          # [128, NJB]
    b1r = b1.reshape(NJB, 128).T
    b2r = b2.reshape(NTB, 128).T                          # [128, NTB]
    par = position_alpha.reshape(NTB, 128).T
    prm = np.ascontiguousarray(
        np.concatenate([s1r, b1r, b2r, par], axis=1))     # [128, 2NJB+2NTB]
    w2t = np.ascontiguousarray(w2.T.reshape(NJB, 128, T)).astype(ml_dtypes.bfloat16)
    ut = np.triu(np.ones((128, 128), dtype=np.float32)).astype(ml_dtypes.bfloat16)
    # LJ[j][p, i] = 1 iff i > j: z_j's colsum feeds prefixes of later blocks
    lj = np.zeros((128, NTB - 1, NTB), dtype=np.float32)
    for j in range(NTB - 1):
        lj[:, j, j + 1:] = 1.0
    lj = lj.reshape(128, (NTB - 1) * NTB).astype(ml_dtypes.bfloat16)
    x_bf = x.astype(ml_dtypes.bfloat16)

    nc = _get_nc()
    in_maps = []
    for i in range(NCORES):
        in_maps.append({
            "x": np.ascontiguousarray(x_bf[i * BPC:(i + 1) * BPC]),
            "w2t": w2t, "prm": prm, "ut": ut, "lj": lj,
        })
    res = run_bass_kernel_spmd(
        nc, in_maps, core_ids=list(range(NCORES)),
        trace=bool(int(os.environ.get("EMA_TRACE", "0"))),
    )
    LAST_RESULTS = res
    return np.concatenate(
        [r["y"].astype(np.float32) for r in res.results], axis=0)


# revision 15
# speedup vs baseline: 1.2997x; 1.2997x over previous
"""EnhancedEMA Trainium2 kernel (v3).

Math (per batch b):
  m_b   = mean(x[b])                                  (scalar)
  h1    = relu(m_b * rowsum(w1) + b1)                 ([C2]; C2 = T/4 = 512)
  tw    = softmax(h1 @ w2.T + b2)                     ([T])
  a_t   = position_alpha[t] * tw[t]
  s_0   = x[b,0]; s_t = a_t x_t + (1-a_t) s_{t-1}

Linearized EMA: c_t = prod_{j<=t}(1-a_j), g_t = a_t/c_t, Q_t = sum g_k x_k,
y_t = c_t * (x0 + Q_t).  sum_t a_t = 0.3 so c stays in [~0.7, 1].

v3 structure (per core, BPC=4 batches, column layout [t_lo=128, ...]):
  - z = g*x into separate z tiles (in-place engine ops run ~14x slow!);
    DVE/Pool split (GpSimd cannot read PSUM, so it only gets SBUF work).
    Row 0 gets the carry fused in: z[0,:] = g_0*x_0 + R_tb (one stt op).
  - Running prefixes R_tb = x0 + sum_{j<tb} colsum(z_j) for ALL 16 blocks
    computed into one [16,512] PSUM tile by 16 accumulating matmuls with
    static prefix-pattern weights (LJ[j][p,i] = 1 iff i > j), plus a
    rank-1 ones matmul adding x0 to every row.
    This removes the serial carry chain: every scan matmul is independent.
  - Carry: z_tb[0,:] += R_tb (bf16 SBUF add, 2x DVE mode).
  - Scan: ps = U.T @ z_tb (inclusive cumsum including R in row 0) so
    y = c * ps is a single ACT scale PSUM->bf16.
  - y written as bf16, widened to f32 on host (tolerance budget 2e-2).
  - Batch means: b0/b1 on PE (ones-matmul, hides in DMA-in shadow),
    b2/b3 via DVE free-dim reduce + GpSimd partition all-reduce.
  - Temporal-attention matmuls paired across 2 batches (rhs = 2 h1 cols).
  - DMA: 32 in / 32 out instructions; y-DMAs issued from the Pool sequencer
    (25ns vs 565ns on sync).
"""

import os
import numpy as np
import ml_dtypes

B, T, C = 32, 2048, 512
NCORES = 8
BPC = B // NCORES        # batches per core
NTB = T // 128           # 16 t-blocks
C2 = T // 4              # 512 hidden
NJB = C2 // 128          # 4 j-blocks

_CACHE = {}
LAST_RESULTS = None


def _build_bass():
    import concourse.bass as bass
    import concourse.bacc as bacc
    import concourse.tile as tile
    from concourse import mybir, bass_isa
    from contextlib import ExitStack

    f32 = mybir.dt.float32
    bf16 = mybir.dt.bfloat16
    AF = mybir.ActivationFunctionType
    ALU = mybir.AluOpType
    AX = mybir.AxisListType

    nc = bacc.Bacc("TRN2", target_bir_lowering=False)

    x_d = nc.dram_tensor("x", [BPC, T, C], bf16, kind="ExternalInput")
    w2t_d = nc.dram_tensor("w2t", [NJB, 128, T], bf16, kind="ExternalInput")
    prm_d = nc.dram_tensor("prm", [128, 2 * NJB + 2 * NTB], f32,
                           kind="ExternalInput")
    u_d = nc.dram_tensor("ut", [128, 128], bf16, kind="ExternalInput")
    lj_d = nc.dram_tensor("lj", [128, (NTB - 1) * NTB], bf16,
                          kind="ExternalInput")
    y_d = nc.dram_tensor("y", [BPC, T, C], bf16, kind="ExternalOutput")

    with ExitStack() as ctx:
        tc = ctx.enter_context(tile.TileContext(nc))
        consts = ctx.enter_context(tc.tile_pool(name="consts", bufs=1))
        xpool = ctx.enter_context(tc.tile_pool(name="xpool", bufs=BPC))
        zpool = ctx.enter_context(tc.tile_pool(name="zpool", bufs=3))
        ypool = ctx.enter_context(tc.tile_pool(name="ypool", bufs=3))
        rpool = ctx.enter_context(tc.tile_pool(name="rpool", bufs=4))
        small = ctx.enter_context(tc.tile_pool(name="small", bufs=4))
        coef = ctx.enter_context(tc.tile_pool(name="coef", bufs=4))
        pscan = ctx.enter_context(tc.tile_pool(name="pscan", bufs=3, space="PSUM"))
        psS = ctx.enter_context(tc.tile_pool(name="psS", bufs=2, space="PSUM"))
        pmean = ctx.enter_context(tc.tile_pool(name="pmean", bufs=1, space="PSUM"))
        psmall = ctx.enter_context(tc.tile_pool(name="psmall", bufs=2, space="PSUM"))

        # ---- constants ----
        NPRM = 2 * NJB + 2 * NTB
        u_sb = consts.tile([128, 128], bf16, name="u_sb")
        nc.sync.dma_start(out=u_sb, in_=u_d[:, :])
        lj_sb = consts.tile([128, (NTB - 1) * NTB], bf16, name="lj_sb")
        nc.sync.dma_start(out=lj_sb, in_=lj_d[:, :])
        w2t_sb = consts.tile([128, NJB, T], bf16, name="w2t_sb")
        nc.sync.dma_start(
            out=w2t_sb,
            in_=bass.AP(tensor=w2t_d[0].tensor, offset=0,
                        ap=[[T, 128], [128 * T, NJB], [1, T]]))
        prm_sb = consts.tile([128, NPRM], f32, name="prm_sb")
        nc.sync.dma_start(out=prm_sb, in_=prm_d[:, :])
        s1_sb = prm_sb[:, 0:NJB]
        b1_sb = prm_sb[:, NJB:2 * NJB]
        b2_sb = prm_sb[:, 2 * NJB:2 * NJB + NTB]
        pa_sb = prm_sb[:, 2 * NJB + NTB:2 * NJB + 2 * NTB]
        onesmat = consts.tile([128, 128], bf16, name="onesmat")
        nc.vector.memset(onesmat, 1.0)
        zeros16 = consts.tile([1, NTB], f32, name="zeros16")
        nc.vector.memset(zeros16, 0.0)

        # ---- stage all x loads (8 chunk DMAs per batch, 2 tb each) ----
        xbs = []
        for b in range(BPC):
            xb = xpool.tile([128, NTB * C], bf16, name=f"xb{b}", tag="xb")
            for q in range(8):
                src = x_d[b, q * 2 * 128:(q + 1) * 2 * 128, :]
                nc.sync.dma_start(
                    out=xb[:, q * 2 * C:(q + 1) * 2 * C],
                    in_=bass.AP(tensor=src.tensor, offset=src.offset,
                                ap=[[C, 128], [128 * C, 2], [1, C]]))
            xbs.append(xb)

        # ---- batch means: sm_b replicated on all partitions ----
        sms = []
        for b in range(BPC):
            sm = small.tile([128, 1], f32, name=f"sm{b}", tag="sm")
            if b < 2:
                pm = pmean.tile([128, C], f32, name=f"pm{b}", tag="pm")
                for tb in range(NTB):
                    nc.tensor.matmul(pm, lhsT=onesmat,
                                     rhs=xbs[b][:, tb * C:(tb + 1) * C],
                                     start=(tb == 0), stop=(tb == NTB - 1))
                smr = small.tile([128, 1], f32, name=f"smr{b}", tag="smr")
                nc.vector.reduce_sum(out=smr, in_=pm, axis=AX.X)
                nc.scalar.mul(sm, smr, 1.0 / float(T * C))
            else:
                rsum = small.tile([128, 1], f32, name=f"rsum{b}", tag="smr")
                nc.vector.reduce_sum(out=rsum, in_=xbs[b], axis=AX.X)
                tot = small.tile([128, 1], f32, name=f"tot{b}", tag="tot")
                nc.gpsimd.partition_all_reduce(
                    tot, rsum, channels=128, reduce_op=bass_isa.ReduceOp.add)
                nc.scalar.mul(sm, tot, 1.0 / float(T * C))
            sms.append(sm)

        for pair in range(BPC // 2):
            b0, b1 = 2 * pair, 2 * pair + 1

            # ---- phase B (paired): h1 -> logits for both batches ----
            h1t2 = small.tile([128, 2 * NJB], bf16, name=f"h1t2{pair}",
                              tag="h1t2")
            for bi, b in enumerate((b0, b1)):
                t1 = small.tile([128, NJB], f32, name=f"t1{b}", tag="t1")
                nc.vector.scalar_tensor_tensor(
                    out=t1, in0=s1_sb, scalar=sms[b], in1=b1_sb,
                    op0=ALU.mult, op1=ALU.add)
                nc.scalar.activation(h1t2[:, bi * NJB:(bi + 1) * NJB], t1,
                                     AF.Relu)

            plb2 = psmall.tile([128, 2 * NTB], f32, name=f"plb2{pair}",
                               tag="ps")
            for tb in range(NTB):
                for jb in range(NJB):
                    nc.tensor.matmul(
                        plb2[:, tb * 2:tb * 2 + 2],
                        lhsT=w2t_sb[:, jb, tb * 128:(tb + 1) * 128],
                        rhs=h1t2[:, jb::NJB],
                        start=(jb == 0),
                        stop=(jb == NJB - 1),
                    )

            # ---- phase C per batch: softmax -> a; c = exp(cumsum ln(1-a)) ----
            gcols, ccols = {}, {}
            for bi, b in enumerate((b0, b1)):
                lg = coef.tile([128, NTB], f32, name=f"lg{b}", tag="lg")
                nc.vector.tensor_add(lg, plb2[:, bi::2], b2_sb)
                e = coef.tile([128, NTB], bf16, name=f"e{b}", tag="e")
                nc.scalar.activation(e, lg, AF.Exp)
                pse = psmall.tile([128, NTB], f32, name=f"pse{b}", tag="ps")
                nc.tensor.matmul(pse, lhsT=onesmat, rhs=e, start=True,
                                 stop=True)
                esumt = small.tile([128, 1], f32, name=f"esumt{b}",
                                   tag="esumt")
                nc.vector.reduce_sum(out=esumt, in_=pse, axis=AX.X)
                rb = small.tile([128, 1], f32, name=f"rb{b}", tag="rb")
                nc.vector.reciprocal(rb, esumt)
                acol = coef.tile([128, NTB], f32, name=f"acol{b}", tag="acol")
                nc.vector.scalar_tensor_tensor(
                    out=acol, in0=e, scalar=rb, in1=pa_sb,
                    op0=ALU.mult, op1=ALU.mult)

                Lb = coef.tile([128, NTB], bf16, name=f"Lb{b}", tag="Lb")
                nc.scalar.activation(Lb, acol, AF.Ln, bias=1.0, scale=-1.0)
                nc.scalar.mul(Lb[0:1, 0:1], Lb[0:1, 0:1], 0.0)
                pT = psmall.tile([128, NTB], f32, name=f"pT{b}", tag="ps")
                nc.tensor.matmul(pT, lhsT=onesmat, rhs=Lb, start=True,
                                 stop=True)
                Tsb = small.tile([1, NTB], f32, name=f"Tsb{b}", tag="Tsb")
                nc.scalar.copy(Tsb, pT[0:1, :])
                stage = small.tile([128, NTB + 1], bf16, name=f"stage{b}",
                                   tag="stage")
                nc.vector.memset(stage, 0.0)
                nc.vector.tensor_tensor_scan(
                    out=stage[0:1, 1:NTB + 1], data0=Tsb, data1=zeros16,
                    initial=0.0, op0=ALU.add, op1=ALU.add,
                )
                pcs = psmall.tile([128, NTB], f32, name=f"pcs{b}", tag="ps")
                nc.tensor.matmul(pcs, lhsT=u_sb, rhs=Lb, start=True,
                                 stop=False)
                nc.tensor.matmul(pcs, lhsT=onesmat, rhs=stage[:, 0:NTB],
                                 start=False, stop=True)
                ccol = coef.tile([128, NTB], f32, name=f"ccol{b}", tag="ccol")
                nc.scalar.activation(ccol, pcs, AF.Exp)
                rccol = coef.tile([128, NTB], f32, name=f"rccol{b}",
                                  tag="rccol")
                nc.scalar.activation(rccol, pcs, AF.Exp, scale=-1.0)
                gcol = coef.tile([128, NTB], f32, name=f"gcol{b}", tag="gcol")
                nc.vector.tensor_mul(gcol, acol, rccol)
                nc.vector.memset(gcol[0:1, 0:1], 0.0)
                gcols[b], ccols[b] = gcol, ccol

            # ---- phase D-1 per batch: z in place, block sums S, prefixes R,
            #      carry rows into z ----
            zs = {}
            for b in (b0, b1):
                xb = xbs[b]
                zb = zpool.tile([128, NTB * C], bf16, name=f"z{b}", tag="z")
                zs[b] = zb
                for tb in range(NTB):
                    gsc = gcols[b][:, tb:tb + 1]
                    if tb % 2 == 0:
                        nc.vector.tensor_scalar_mul(
                            zb[:, tb * C:(tb + 1) * C],
                            in0=xb[:, tb * C:(tb + 1) * C], scalar1=gsc)
                    else:
                        nc.gpsimd.tensor_scalar_mul(
                            zb[:, tb * C:(tb + 1) * C],
                            in0=xb[:, tb * C:(tb + 1) * C], scalar1=gsc)
                # R[i] = x0 + sum_{j<i} colsum(z_j), all 16 rows at once
                Rall = psS.tile([16, C], f32, name=f"Rall{b}", tag="SR")
                nc.tensor.matmul(Rall, lhsT=onesmat[0:1, 0:NTB],
                                 rhs=xb[0:1, 0:C], start=True, stop=False)
                for j in range(NTB - 1):
                    nc.tensor.matmul(Rall,
                                     lhsT=lj_sb[:, j * NTB:(j + 1) * NTB],
                                     rhs=zb[:, j * C:(j + 1) * C],
                                     start=False, stop=(j == NTB - 2))
                Rsb = rpool.tile([16, C], bf16, name=f"Rsb{b}", tag="Rsb")
                nc.vector.tensor_scalar_add(Rsb, in0=Rall, scalar1=0.0)
                # engines cannot address partition tb directly: flatten the
                # 16 R rows onto partition 0 with a tiny SBUF->SBUF DMA
                rrow = rpool.tile([1, NTB * C], bf16, name=f"rrow{b}",
                                  tag="rrow")
                nc.gpsimd.dma_start(out=rrow, in_=Rsb)
                # overwrite z row 0 with g_0*x_0 + R_tb (out-of-place stt)
                for tb in range(NTB):
                    nc.vector.scalar_tensor_tensor(
                        out=zb[0:1, tb * C:(tb + 1) * C],
                        in0=xb[0:1, tb * C:(tb + 1) * C],
                        scalar=gcols[b][0:1, tb:tb + 1],
                        in1=rrow[0:1, tb * C:(tb + 1) * C],
                        op0=ALU.mult, op1=ALU.add)

            # ---- phase D-2 (tb-major): scan matmuls + y scale + out DMA ----
            for tb in range(NTB):
                y2 = ypool.tile([128, 2 * C], bf16, name=f"y{pair}_{tb}",
                                tag="y")
                for bi, b in enumerate((b0, b1)):
                    ps = pscan.tile([128, C], f32, name=f"psQ{b}_{tb}",
                                    tag="s")
                    nc.tensor.matmul(ps, lhsT=u_sb,
                                     rhs=zs[b][:, tb * C:(tb + 1) * C],
                                     start=True, stop=True)
                    nc.scalar.mul(y2[:, bi * C:(bi + 1) * C], ps,
                                  ccols[b][:, tb:tb + 1])
                dst = y_d[b0, tb * 128:(tb + 1) * 128, :]
                nc.gpsimd.dma_start(
                    out=bass.AP(tensor=dst.tensor, offset=dst.offset,
                                ap=[[C, 128], [T * C, 2], [1, C]]),
                    in_=y2)

    nc.compile()
    return nc


def _get_nc():
    if "nc" not in _CACHE:
        _CACHE["nc"] = _build_bass()
    return _CACHE["nc"]


def kernel(x, position_alpha, w1, b1, w2, b2):
    global LAST_RESULTS
    from concourse.bass_utils import run_bass_kernel_spmd

    x = np.asarray(x, dtype=np.float32)
    position_alpha = np.asarray(position_alpha, dtype=np.float32)
    w1 = np.asarray(w1, dtype=np.float32)
    b1 = np.asarray(b1, dtype=np.float32)
    w2 = np.asarray(w2, dtype=np.float32)
    b2 = np.asarray(b2, dtype=np.float32)

    # host-side parameter prep (layout only / trivial reductions)
    s1 = w1.sum(axis=1)                                   # [C2]
    s1r = s1.reshape(NJB, 128).T                          # [128, NJB]
    b1r = b1.reshape(NJB, 128).T
    b2r = b2.reshape(NTB, 128).T                          # [128, NTB]
    par = position_alpha.reshape(NTB, 128).T
    prm = np.ascontiguousarray(
        np.concatenate([s1r, b1r, b2r, par], axis=1))     # [128, 2NJB+2NTB]
    w2t = np.ascontiguousarray(w2.T.reshape(NJB, 128, T)).astype(ml_dtypes.bfloat16)
    ut = np.triu(np.ones((128, 128), dtype=np.float32)).astype(ml_dtypes.bfloat16)
    # LJ[j][p, i] = 1 iff i > j: z_j's colsum feeds prefixes of later blocks
    lj = np.zeros((128, NTB - 1, NTB), dtype=np.float32)
    for j in range(NTB - 1):
        lj[:, j, j + 1:] = 1.0
    lj = lj.reshape(128, (NTB - 1) * NTB).astype(ml_dtypes.bfloat16)
    x_bf = x.astype(ml_dtypes.bfloat16)

    nc = _get_nc()
    in_maps = []
    for i in range(NCORES):
        in_maps.append({
            "x": np.ascontiguousarray(x_bf[i * BPC:(i + 1) * BPC]),
            "w2t": w2t, "prm": prm, "ut": ut, "lj": lj,
        })
    res = run_bass_kernel_spmd(
        nc, in_maps, core_ids=list(range(NCORES)),
        trace=bool(int(os.environ.get("EMA_TRACE", "0"))),
    )
    LAST_RESULTS = res
    return np.concatenate(
        [r["y"].astype(np.float32) for r in res.results], axis=0)


# revision 18
# speedup vs baseline: 3.0603x; 2.3547x over previous
"""EnhancedEMA Trainium2 kernel (v3).

Math (per batch b):
  m_b   = mean(x[b])                                  (scalar)
  h1    = relu(m_b * rowsum(w1) + b1)                 ([C2]; C2 = T/4 = 512)
  tw    = softmax(h1 @ w2.T + b2)                     ([T])
  a_t   = position_alpha[t] * tw[t]
  s_0   = x[b,0]; s_t = a_t x_t + (1-a_t) s_{t-1}

Linearized EMA: c_t = prod_{j<=t}(1-a_j), g_t = a_t/c_t, Q_t = sum g_k x_k,
y_t = c_t * (x0 + Q_t).  sum_t a_t = 0.3 so c stays in [~0.7, 1].

v3/v4 structure (per core, BPC=4 batches, column layout [t_lo=128, ...]):
  Measured engine facts driving the design: in-place engine ops run ~14x
  slow; InstTensorScalar with an AP scalar is ~4.5 cyc/elem on DVE and ~14
  on GpSimd (useless), but DVE scalar_tensor_tensor (mult+add) is ~1.4
  cyc/elem and ACT activation-with-scale ~1.7; GpSimd cannot touch PSUM;
  engine operands must start at partition 0/32/64/96.
  - z = g*x into fresh z tiles: DVE stt (x*g)+0 and ACT Copy(x*scale),
    split between the two engines.
  - Running prefixes R_tb = x0 + sum_{j<tb} colsum(z_j) for ALL 16 blocks
    via 16 accumulating matmuls into one [16,512] PSUM tile with static
    prefix-pattern weights (LJ[j][p,i] = 1 iff i > j) + rank-1 x0 matmul.
    No serial carry chain: every scan matmul is independent.
  - Carry fix-up without per-block row ops: ONE [16,512] stt computes
    zfix[tb] = g_t0*x_t0 + R_tb in the partition-major domain (x row-0
    values gathered by a tiny SBUF->SBUF DMA), then one DMA scatters zfix
    into z row 0 of all 16 blocks.
  - Scan: ps = U.T @ z_tb (inclusive cumsum, R arrives via z row 0) so
    y = c * ps is a single scale PSUM->bf16, split ACT / DVE-stt.
  - y written as bf16, widened to f32 on host (tolerance budget 2e-2).
  - Batch means: b0/b1 on PE (ones-matmul, hides in DMA-in shadow),
    b2/b3 via DVE free-dim reduce + GpSimd partition all-reduce.
  - Temporal-attention matmuls paired across 2 batches (rhs = 2 h1 cols).
  - DMA: 32 in / 32 out instructions; y-DMAs issued from the Pool sequencer
    (25ns vs 565ns on sync).
"""

import os
import numpy as np
import ml_dtypes

B, T, C = 32, 2048, 512
NCORES = 8
BPC = B // NCORES        # batches per core
NTB = T // 128           # 16 t-blocks
C2 = T // 4              # 512 hidden
NJB = C2 // 128          # 4 j-blocks

_CACHE = {}
LAST_RESULTS = None


def _build_bass():
    import concourse.bass as bass
    import concourse.bacc as bacc
    import concourse.tile as tile
    from concourse import mybir, bass_isa
    from contextlib import ExitStack

    f32 = mybir.dt.float32
    bf16 = mybir.dt.bfloat16
    AF = mybir.ActivationFunctionType
    ALU = mybir.AluOpType
    AX = mybir.AxisListType

    nc = bacc.Bacc("TRN2", target_bir_lowering=False)

    x_d = nc.dram_tensor("x", [BPC, T, C], bf16, kind="ExternalInput")
    w2t_d = nc.dram_tensor("w2t", [NJB, 128, T], bf16, kind="ExternalInput")
    prm_d = nc.dram_tensor("prm", [128, 2 * NJB + 2 * NTB], f32,
                           kind="ExternalInput")
    u_d = nc.dram_tensor("ut", [128, 128], bf16, kind="ExternalInput")
    lj_d = nc.dram_tensor("lj", [128, (NTB - 1) * NTB], bf16,
                          kind="ExternalInput")
    y_d = nc.dram_tensor("y", [BPC, T, C], bf16, kind="ExternalOutput")

    with ExitStack() as ctx:
        tc = ctx.enter_context(tile.TileContext(nc))
        consts = ctx.enter_context(tc.tile_pool(name="consts", bufs=1))
        xpool = ctx.enter_context(tc.tile_pool(name="xpool", bufs=BPC))
        zpool = ctx.enter_context(tc.tile_pool(name="zpool", bufs=3))
        ypool = ctx.enter_context(tc.tile_pool(name="ypool", bufs=3))
        rpool = ctx.enter_context(tc.tile_pool(name="rpool", bufs=4))
        small = ctx.enter_context(tc.tile_pool(name="small", bufs=4))
        coef = ctx.enter_context(tc.tile_pool(name="coef", bufs=4))
        pscan = ctx.enter_context(tc.tile_pool(name="pscan", bufs=3, space="PSUM"))
        psS = ctx.enter_context(tc.tile_pool(name="psS", bufs=2, space="PSUM"))
        pmean = ctx.enter_context(tc.tile_pool(name="pmean", bufs=1, space="PSUM"))
        psmall = ctx.enter_context(tc.tile_pool(name="psmall", bufs=2, space="PSUM"))

        # ---- constants ----
        NPRM = 2 * NJB + 2 * NTB
        u_sb = consts.tile([128, 128], bf16, name="u_sb")
        nc.sync.dma_start(out=u_sb, in_=u_d[:, :])
        lj_sb = consts.tile([128, (NTB - 1) * NTB], bf16, name="lj_sb")
        nc.sync.dma_start(out=lj_sb, in_=lj_d[:, :])
        w2t_sb = consts.tile([128, NJB, T], bf16, name="w2t_sb")
        nc.sync.dma_start(
            out=w2t_sb,
            in_=bass.AP(tensor=w2t_d[0].tensor, offset=0,
                        ap=[[T, 128], [128 * T, NJB], [1, T]]))
        prm_sb = consts.tile([128, NPRM], f32, name="prm_sb")
        nc.sync.dma_start(out=prm_sb, in_=prm_d[:, :])
        s1_sb = prm_sb[:, 0:NJB]
        b1_sb = prm_sb[:, NJB:2 * NJB]
        b2_sb = prm_sb[:, 2 * NJB:2 * NJB + NTB]
        pa_sb = prm_sb[:, 2 * NJB + NTB:2 * NJB + 2 * NTB]
        onesmat = consts.tile([128, 128], bf16, name="onesmat")
        nc.vector.memset(onesmat, 1.0)
        zeros16 = consts.tile([1, NTB], f32, name="zeros16")
        nc.vector.memset(zeros16, 0.0)
        zerosC = consts.tile([128, C], bf16, name="zerosC")
        nc.vector.memset(zerosC, 0.0)

        # ---- stage all x loads (8 chunk DMAs per batch, 2 tb each) ----
        xbs = []
        for b in range(BPC):
            xb = xpool.tile([128, NTB * C], bf16, name=f"xb{b}", tag="xb")
            for q in range(8):
                src = x_d[b, q * 2 * 128:(q + 1) * 2 * 128, :]
                nc.sync.dma_start(
                    out=xb[:, q * 2 * C:(q + 1) * 2 * C],
                    in_=bass.AP(tensor=src.tensor, offset=src.offset,
                                ap=[[C, 128], [128 * C, 2], [1, C]]))
            xbs.append(xb)

        # ---- batch means: sm_b replicated on all partitions ----
        sms = []
        for b in range(BPC):
            sm = small.tile([128, 1], f32, name=f"sm{b}", tag="sm")
            if b < 2:
                pm = pmean.tile([128, C], f32, name=f"pm{b}", tag="pm")
                for tb in range(NTB):
                    nc.tensor.matmul(pm, lhsT=onesmat,
                                     rhs=xbs[b][:, tb * C:(tb + 1) * C],
                                     start=(tb == 0), stop=(tb == NTB - 1))
                smr = small.tile([128, 1], f32, name=f"smr{b}", tag="smr")
                nc.vector.reduce_sum(out=smr, in_=pm, axis=AX.X)
                nc.scalar.mul(sm, smr, 1.0 / float(T * C))
            else:
                rsum = small.tile([128, 1], f32, name=f"rsum{b}", tag="smr")
                nc.vector.reduce_sum(out=rsum, in_=xbs[b], axis=AX.X)
                tot = small.tile([128, 1], f32, name=f"tot{b}", tag="tot")
                nc.gpsimd.partition_all_reduce(
                    tot, rsum, channels=128, reduce_op=bass_isa.ReduceOp.add)
                nc.scalar.mul(sm, tot, 1.0 / float(T * C))
            sms.append(sm)

        for pair in range(BPC // 2):
            b0, b1 = 2 * pair, 2 * pair + 1

            # ---- phase B (paired): h1 -> logits for both batches ----
            h1t2 = small.tile([128, 2 * NJB], bf16, name=f"h1t2{pair}",
                              tag="h1t2")
            for bi, b in enumerate((b0, b1)):
                t1 = small.tile([128, NJB], f32, name=f"t1{b}", tag="t1")
                nc.vector.scalar_tensor_tensor(
                    out=t1, in0=s1_sb, scalar=sms[b], in1=b1_sb,
                    op0=ALU.mult, op1=ALU.add)
                nc.scalar.activation(h1t2[:, bi * NJB:(bi + 1) * NJB], t1,
                                     AF.Relu)

            plb2 = psmall.tile([128, 2 * NTB], f32, name=f"plb2{pair}",
                               tag="ps")
            for tb in range(NTB):
                for jb in range(NJB):
                    nc.tensor.matmul(
                        plb2[:, tb * 2:tb * 2 + 2],
                        lhsT=w2t_sb[:, jb, tb * 128:(tb + 1) * 128],
                        rhs=h1t2[:, jb::NJB],
                        start=(jb == 0),
                        stop=(jb == NJB - 1),
                    )

            # ---- phase C per batch: softmax -> a; c = exp(cumsum ln(1-a)) ----
            gcols, ccols = {}, {}
            for bi, b in enumerate((b0, b1)):
                lg = coef.tile([128, NTB], f32, name=f"lg{b}", tag="lg")
                nc.vector.tensor_add(lg, plb2[:, bi::2], b2_sb)
                e = coef.tile([128, NTB], bf16, name=f"e{b}", tag="e")
                nc.scalar.activation(e, lg, AF.Exp)
                pse = psmall.tile([128, NTB], f32, name=f"pse{b}", tag="ps")
                nc.tensor.matmul(pse, lhsT=onesmat, rhs=e, start=True,
                                 stop=True)
                esumt = small.tile([128, 1], f32, name=f"esumt{b}",
                                   tag="esumt")
                nc.vector.reduce_sum(out=esumt, in_=pse, axis=AX.X)
                rb = small.tile([128, 1], f32, name=f"rb{b}", tag="rb")
                nc.vector.reciprocal(rb, esumt)
                acol = coef.tile([128, NTB], f32, name=f"acol{b}", tag="acol")
                nc.vector.scalar_tensor_tensor(
                    out=acol, in0=e, scalar=rb, in1=pa_sb,
                    op0=ALU.mult, op1=ALU.mult)

                Lb = coef.tile([128, NTB], bf16, name=f"Lb{b}", tag="Lb")
                nc.scalar.activation(Lb, acol, AF.Ln, bias=1.0, scale=-1.0)
                nc.scalar.mul(Lb[0:1, 0:1], Lb[0:1, 0:1], 0.0)
                pT = psmall.tile([128, NTB], f32, name=f"pT{b}", tag="ps")
                nc.tensor.matmul(pT, lhsT=onesmat, rhs=Lb, start=True,
                                 stop=True)
                Tsb = small.tile([1, NTB], f32, name=f"Tsb{b}", tag="Tsb")
                nc.scalar.copy(Tsb, pT[0:1, :])
                stage = small.tile([128, NTB + 1], bf16, name=f"stage{b}",
                                   tag="stage")
                nc.vector.memset(stage, 0.0)
                nc.vector.tensor_tensor_scan(
                    out=stage[0:1, 1:NTB + 1], data0=Tsb, data1=zeros16,
                    initial=0.0, op0=ALU.add, op1=ALU.add,
                )
                pcs = psmall.tile([128, NTB], f32, name=f"pcs{b}", tag="ps")
                nc.tensor.matmul(pcs, lhsT=u_sb, rhs=Lb, start=True,
                                 stop=False)
                nc.tensor.matmul(pcs, lhsT=onesmat, rhs=stage[:, 0:NTB],
                                 start=False, stop=True)
                ccol = coef.tile([128, NTB], f32, name=f"ccol{b}", tag="ccol")
                nc.scalar.activation(ccol, pcs, AF.Exp)
                rccol = coef.tile([128, NTB], f32, name=f"rccol{b}",
                                  tag="rccol")
                nc.scalar.activation(rccol, pcs, AF.Exp, scale=-1.0)
                gcol = coef.tile([128, NTB], f32, name=f"gcol{b}", tag="gcol")
                nc.vector.tensor_mul(gcol, acol, rccol)
                nc.vector.memset(gcol[0:1, 0:1], 0.0)
                gcols[b], ccols[b] = gcol, ccol

            # ---- phase D-1 per batch: z in place, block sums S, prefixes R,
            #      carry rows into z ----
            zs = {}
            for b in (b0, b1):
                xb = xbs[b]
                # gather x[b, 128*tb, :] rows onto partitions tb (tiny DMAs)
                xfirst = rpool.tile([16, C], bf16, name=f"xf{b}", tag="xf")
                nc.gpsimd.dma_start(out=xfirst, in_=xb[0:1, :])
                gfirst = rpool.tile([16, 1], f32, name=f"gf{b}", tag="gf")
                nc.gpsimd.dma_start(out=gfirst, in_=gcols[b][0:1, :])

                zb = zpool.tile([128, NTB * C], bf16, name=f"z{b}", tag="z")
                zs[b] = zb
                for tb in range(NTB):
                    gsc = gcols[b][:, tb:tb + 1]
                    zsl = zb[:, tb * C:(tb + 1) * C]
                    xsl = xb[:, tb * C:(tb + 1) * C]
                    if tb % 2 == 0:
                        nc.vector.scalar_tensor_tensor(
                            out=zsl, in0=xsl, scalar=gsc, in1=zerosC,
                            op0=ALU.mult, op1=ALU.add)
                    else:
                        nc.scalar.mul(zsl, xsl, gsc)
                # R[i] = x0 + sum_{j<i} colsum(z_j), all 16 rows at once
                Rall = psS.tile([16, C], f32, name=f"Rall{b}", tag="SR")
                nc.tensor.matmul(Rall, lhsT=onesmat[0:1, 0:NTB],
                                 rhs=xb[0:1, 0:C], start=True, stop=False)
                for j in range(NTB - 1):
                    nc.tensor.matmul(Rall,
                                     lhsT=lj_sb[:, j * NTB:(j + 1) * NTB],
                                     rhs=zb[:, j * C:(j + 1) * C],
                                     start=False, stop=(j == NTB - 2))
                # zfix[tb] = g_t0 * x_t0 + R_tb, one [16,C] stt, then one
                # DMA scatters it into z row 0 of every block
                zfix = rpool.tile([16, C], bf16, name=f"zfix{b}", tag="zfix")
                nc.vector.scalar_tensor_tensor(
                    out=zfix, in0=xfirst, scalar=gfirst, in1=Rall,
                    op0=ALU.mult, op1=ALU.add)
                nc.gpsimd.dma_start(out=zb[0:1, :], in_=zfix)

            # ---- phase D-2 (tb-major): scan matmuls + y scale + out DMA ----
            for tb in range(NTB):
                y2 = ypool.tile([128, 2 * C], bf16, name=f"y{pair}_{tb}",
                                tag="y")
                for bi, b in enumerate((b0, b1)):
                    ps = pscan.tile([128, C], f32, name=f"psQ{b}_{tb}",
                                    tag="s")
                    nc.tensor.matmul(ps, lhsT=u_sb,
                                     rhs=zs[b][:, tb * C:(tb + 1) * C],
                                     start=True, stop=True)
                    ysl = y2[:, bi * C:(bi + 1) * C]
                    csc = ccols[b][:, tb:tb + 1]
                    if (tb + bi) % 2 == 0:
                        nc.scalar.mul(ysl, ps, csc)
                    else:
                        nc.vector.scalar_tensor_tensor(
                            out=ysl, in0=ps, scalar=csc, in1=zerosC,
                            op0=ALU.mult, op1=ALU.add)
                dst = y_d[b0, tb * 128:(tb + 1) * 128, :]
                nc.gpsimd.dma_start(
                    out=bass.AP(tensor=dst.tensor, offset=dst.offset,
                                ap=[[C, 128], [T * C, 2], [1, C]]),
                    in_=y2)

    nc.compile()
    return nc


def _get_nc():
    if "nc" not in _CACHE:
        _CACHE["nc"] = _build_bass()
    return _CACHE["nc"]


def kernel(x, position_alpha, w1, b1, w2, b2):
    global LAST_RESULTS
    from concourse.bass_utils import run_bass_kernel_spmd

    x = np.asarray(x, dtype=np.float32)
    position_alpha = np.asarray(position_alpha, dtype=np.float32)
    w1 = np.asarray(w1, dtype=np.float32)
    b1 = np.asarray(b1, dtype=np.float32)
    w2 = np.asarray(w2, dtype=np.float32)
    b2 = np.asarray(b2, dtype=np.float32)

    # host-side parameter prep (layout only / trivial reductions)
    s1 = w1.sum(axis=1)                                   # [C2]
    s1r = s1.reshape(NJB, 128).T                          # [128, NJB]
    b1r = b1.reshape(NJB, 128).T
    b2r = b2.reshape(NTB, 128).T                          # [128, NTB]
    par = position_alpha.reshape(NTB, 128).T
    prm = np.ascontiguousarray(
        np.concatenate([s1r, b1r, b2r, par], axis=1))     # [128, 2NJB+2NTB]
    w2t = np.ascontiguousarray(w2.T.reshape(NJB, 128, T)).astype(ml_dtypes.bfloat16)
    ut = np.triu(np.ones((128, 128), dtype=np.float32)).astype(ml_dtypes.bfloat16)
    # LJ[j][p, i] = 1 iff i > j: z_j's colsum feeds prefixes of later blocks
    lj = np.zeros((128, NTB - 1, NTB), dtype=np.float32)
    for j in range(NTB - 1):
        lj[:, j, j + 1:] = 1.0
    lj = lj.reshape(128, (NTB - 1) * NTB).astype(ml_dtypes.bfloat16)
    x_bf = x.astype(ml_dtypes.bfloat16)

    nc = _get_nc()
    in_maps = []
    for i in range(NCORES):
        in_maps.append({
            "x": np.ascontiguousarray(x_bf[i * BPC:(i + 1) * BPC]),
            "w2t": w2t, "prm": prm, "ut": ut, "lj": lj,
        })
    res = run_bass_kernel_spmd(
        nc, in_maps, core_ids=list(range(NCORES)),
        trace=bool(int(os.environ.get("EMA_TRACE", "0"))),
    )
    LAST_RESULTS = res
    return np.concatenate(
        [r["y"].astype(np.float32) for r in res.results], axis=0)
